# revision 16
# baseline (speedup 1.0000x reference)
"""Trainium2 Bass kernel for the CVOnly RNN problem.

Computes h_last of a single-layer tanh RNN (hidden_size H=2) over
cv: [B=4096, T=512, D=64], returning [B, 2]:

    xw   = cv @ W_ih.T + b_ih + b_hh          # [B, T, 2]
    h_t  = tanh(xw[:, t] + h_{t-1} @ W_hh.T)  # scan over T
    out  = h_T

Sharding: pure data-parallel over batch; each of the 8 cores handles 512
batch rows, RNN weights replicated.

Per-core design:
  - Host pre-packs the cv shard into [tblk=128, part=128, free=1024] f32
    where partition = (g_loc, d) and free = (tq, pair, b_lo): each
    [128, 1024] block is a fully contiguous 512KB DMA.
  - Per time-step t, four f32 matmuls with block-diagonal copies of
    W_ih.T (contraction over (g_loc, d) = 128) produce the input
    projection for all 512 batch rows as a PSUM tile
    [16 = (g, h), 64 = b_lo] (g = 8 groups of 64 batch rows).
  - A fifth tiny fp16 matmul (single pass, vs two half-rate passes for
    f32) accumulates W_hh @ h_{t-1} into the same PSUM bank via a
    block-diagonal fp16 W_hh.T against the fp16 state tile.
  - ScalarE computes h_t = tanh(psum + bias) with a per-partition f32
    bias absorbing b_ih + b_hh, writing the fp16 state tile (fp16 out
    halves the ScalarE per-element cost; FD=64 keeps the op short since
    this op sits on the 512-step serial chain).
  The xw matmuls are emitted AHEAD steps early so only mix-matmul + tanh
  are on the sequential dependence chain. Keeping cv / W_ih in f32 keeps
  the dominant error term at fp32 level; fp16 state/W_hh contributes
  ~6e-4 absmax (verified against the exact recurrence numerically).
"""

import os
import numpy as np

B, T, D = 4096, 512, 64
H = 2
N_CORES = 8
B_CORE = B // N_CORES  # 512
NG = 8                 # batch groups per core
BL = 64                # b_lo within a group
NP = 2 * NG            # state partitions (g, h) = 16
NPAIR = 4              # g-pairs -> xw matmuls per step
TQ = 4                 # time-steps per DMA block
AHEAD = 4              # xw pipeline depth; psum pool (7) leaves a 2-slot cushion
CV_BUFS = 20           # SBUF staging buffers of 512KB each

LAST_EXEC_TIME_NS = None
LAST_RESULT = None

_PROGRAM_CACHE = {}


def _build_program(t_steps):
    from concourse import bacc, tile
    import concourse.mybir as mybir

    f32 = mybir.dt.float32
    f16 = mybir.dt.float16
    ntblk = t_steps // TQ
    fwidth = TQ * NPAIR * BL  # 1024

    nc = bacc.Bacc()
    cvr = nc.declare_dram_parameter("cvr", [ntblk, 128, fwidth], f32, isOutput=False)
    ls = [nc.declare_dram_parameter(f"l{p}", [128, NP], f32, isOutput=False)
          for p in range(NPAIR)]
    wb = nc.declare_dram_parameter("wb", [NP, NP], f16, isOutput=False)
    bias = nc.declare_dram_parameter("bias", [NP, 1], f32, isOutput=False)
    hout = nc.declare_dram_parameter("hout", [NP, BL], f16, isOutput=True)

    with tile.TileContext(nc) as tc:
        with tc.tile_pool(name="const", bufs=1) as cpool, \
             tc.tile_pool(name="cv", bufs=min(CV_BUFS, ntblk)) as cvpool, \
             tc.tile_pool(name="state", bufs=t_steps + 8) as spool, \
             tc.tile_pool(name="scps", bufs=1, space="PSUM") as scps_pool, \
             tc.tile_pool(name="ps", bufs=7, space="PSUM") as ppool:
            l_t = []
            for p in range(NPAIR):
                lt = cpool.tile([128, NP], f32, tag=f"l{p}")
                nc.sync.dma_start(out=lt[:], in_=ls[p][:])
                l_t.append(lt)
            wb_t = cpool.tile([NP, NP], f16)
            nc.sync.dma_start(out=wb_t[:], in_=wb[:])
            bias_t = cpool.tile([NP, 1], f32)
            nc.sync.dma_start(out=bias_t[:], in_=bias[:])

            # Prologue: absorb each const-DMA semaphore with a dummy op so
            # later matmuls don't accumulate multiple sync waits.
            scratch_ps = scps_pool.tile([NP, NP], f32)
            for p in range(NPAIR):
                nc.tensor.matmul(scratch_ps[:], l_t[p][:NP, :], l_t[p][:NP, :],
                                 start=True, stop=True)
            nc.tensor.matmul(scratch_ps[:], wb_t[:], wb_t[:],
                             start=True, stop=True)
            scratch_sb = cpool.tile([NP, 1], f32)
            nc.scalar.activation(
                scratch_sb[:], bias_t[:], mybir.ActivationFunctionType.Tanh,
                bias=bias_t[:], scale=1.0,
            )

            # HAM warmup: ~6us of dummy matmuls while the first cv DMAs are
            # in flight, so the PE clock-gate is at 8/8 (2.4 GHz) when the
            # real loop starts (saves the ~4us cold-start penalty).
            for _ in range(60):
                nc.tensor.matmul(scratch_ps[:], l_t[0][:], l_t[0][:, :NP],
                                 start=True, stop=True)

            cvmap = {}
            psq = {}
            state_prev = None
            for i in range(t_steps + AHEAD):
                if i < t_steps:
                    tblk, tq = divmod(i, TQ)
                    if tq == 0:
                        cv_tile = cvpool.tile([128, fwidth], f32)
                        nc.sync.dma_start(out=cv_tile[:], in_=cvr[tblk])
                        cvmap[tblk] = cv_tile
                    ps = ppool.tile([NP, BL], f32)
                    psq[i] = ps
                    base = tq * NPAIR * BL
                    for p in range(NPAIR):
                        nc.tensor.matmul(
                            ps[:], l_t[p][:],
                            cvmap[tblk][:, base + p * BL:base + (p + 1) * BL],
                            start=(p == 0), stop=(i == 0 and p == NPAIR - 1),
                        )
                s = i - AHEAD
                if s >= 0:
                    ps = psq.pop(s)
                    if s > 0:
                        nc.tensor.matmul(
                            ps[:], wb_t[:], state_prev[:],
                            start=False, stop=True,
                        )
                    st = spool.tile([NP, BL], f16)
                    nc.scalar.activation(
                        st[:], ps[:], mybir.ActivationFunctionType.Tanh,
                        bias=bias_t[:], scale=1.0,
                    )
                    state_prev = st
            nc.sync.dma_start(out=hout[:], in_=state_prev[:])
    nc.compile()
    return nc


def _pack_weights(W_ih, W_hh, b_ih, b_hh):
    Ls = []
    for p in range(NPAIR):
        L = np.zeros((128, NP), dtype=np.float32)
        for gl in range(2):
            g = 2 * p + gl
            for h in range(H):
                L[gl * 64:(gl + 1) * 64, g * 2 + h] = W_ih[h, :]
        Ls.append(L)
    WB = np.zeros((NP, NP), dtype=np.float16)
    w16 = W_hh.astype(np.float16)
    for g in range(NG):
        for h in range(H):
            for j in range(H):
                WB[g * 2 + h, g * 2 + j] = w16[j, h]
    biasv = np.tile((b_ih + b_hh).astype(np.float32), NG).reshape(NP, 1)
    return Ls, WB, np.ascontiguousarray(biasv)


def _pack_cv(cv, t_steps):
    # cv: [B, T, D] -> [core, tblk, (g_loc, d), (tq, pair, b_lo)]
    # b_local = pair*128 + g_loc*64 + b_lo
    ntblk = t_steps // TQ
    cv6 = cv.reshape(N_CORES, NPAIR, 2, BL, ntblk, TQ, D)  # core,p,gl,blo,tblk,tq,d
    cvR = cv6.transpose(0, 4, 2, 6, 5, 1, 3)               # core,tblk,gl,d,tq,p,blo
    return np.ascontiguousarray(
        cvR.reshape(N_CORES, ntblk, 128, TQ * NPAIR * BL))


def kernel(x=None, cv=None, W_ih=None, W_hh=None, b_ih=None, b_hh=None, **_):
    global LAST_EXEC_TIME_NS, LAST_RESULT
    from concourse.bass_utils import run_bass_kernel_spmd

    cv = np.ascontiguousarray(cv, dtype=np.float32)
    t_steps = cv.shape[1]
    if t_steps not in _PROGRAM_CACHE:
        _PROGRAM_CACHE[t_steps] = _build_program(t_steps)
    nc = _PROGRAM_CACHE[t_steps]

    Ls, WB, biasv = _pack_weights(
        np.asarray(W_ih, dtype=np.float32), np.asarray(W_hh, dtype=np.float32),
        np.asarray(b_ih, dtype=np.float32), np.asarray(b_hh, dtype=np.float32))
    cvR = _pack_cv(cv, t_steps)

    in_maps = [
        {"cvr": cvR[c], "wb": WB, "bias": biasv,
         **{f"l{p}": Ls[p] for p in range(NPAIR)}}
        for c in range(N_CORES)
    ]
    trace = bool(int(os.environ.get("KERNEL_TRACE", "0")))
    res = run_bass_kernel_spmd(nc, in_maps, list(range(N_CORES)), trace=trace)
    LAST_EXEC_TIME_NS = res.exec_time_ns
    LAST_RESULT = res

    out = np.empty((B, H), dtype=np.float32)
    for c in range(N_CORES):
        hc = res.results[c]["hout"].astype(np.float32)  # [(g,h)=16, b_lo=64]
        out[c * B_CORE:(c + 1) * B_CORE] = (
            hc.reshape(NG, H, BL).transpose(0, 2, 1).reshape(B_CORE, H)
        )
    return out


# revision 17
# speedup vs baseline: 1.0037x; 1.0037x over previous
"""Trainium2 Bass kernel for the CVOnly RNN problem.

Computes h_last of a single-layer tanh RNN (hidden_size H=2) over
cv: [B=4096, T=512, D=64], returning [B, 2]:

    xw   = cv @ W_ih.T + b_ih + b_hh          # [B, T, 2]
    h_t  = tanh(xw[:, t] + h_{t-1} @ W_hh.T)  # scan over T
    out  = h_T

Sharding: pure data-parallel over batch; each of the 8 cores handles 512
batch rows, RNN weights replicated.

Per-core design:
  - Host pre-packs the cv shard into [tblk=128, part=128, free=1024] f32
    where partition = (g_loc, d) and free = (tq, pair, b_lo): each
    [128, 1024] block is a fully contiguous 512KB DMA.
  - Per time-step t, four f32 matmuls with block-diagonal copies of
    W_ih.T (contraction over (g_loc, d) = 128) produce the input
    projection for all 512 batch rows as a PSUM tile
    [16 = (g, h), 64 = b_lo] (g = 8 groups of 64 batch rows).
  - A fifth tiny fp16 matmul (single pass, vs two half-rate passes for
    f32) accumulates W_hh @ h_{t-1} into the same PSUM bank via a
    block-diagonal fp16 W_hh.T against the fp16 state tile.
  - ScalarE computes h_t = tanh(psum + bias) with a per-partition f32
    bias absorbing b_ih + b_hh, writing the fp16 state tile (fp16 out
    halves the ScalarE per-element cost; FD=64 keeps the op short since
    this op sits on the 512-step serial chain).
  The xw matmuls are emitted AHEAD steps early so only mix-matmul + tanh
  are on the sequential dependence chain. Keeping cv / W_ih in f32 keeps
  the dominant error term at fp32 level; fp16 state/W_hh contributes
  ~6e-4 absmax (verified against the exact recurrence numerically).
"""

import os
import numpy as np

B, T, D = 4096, 512, 64
H = 2
N_CORES = 8
B_CORE = B // N_CORES  # 512
NG = 8                 # batch groups per core
BL = 64                # b_lo within a group
NP = 2 * NG            # state partitions (g, h) = 16
NPAIR = 4              # g-pairs -> xw matmuls per step
TQ = 4                 # time-steps per DMA block
AHEAD = 4              # xw pipeline depth; psum pool (7) leaves a 2-slot cushion
CV_BUFS = 20           # SBUF staging buffers of 512KB each

LAST_EXEC_TIME_NS = None
LAST_RESULT = None

_PROGRAM_CACHE = {}


def _build_program(t_steps):
    from concourse import bacc, tile
    import concourse.mybir as mybir

    f32 = mybir.dt.float32
    f16 = mybir.dt.float16
    ntblk = t_steps // TQ
    fwidth = TQ * NPAIR * BL  # 1024

    nc = bacc.Bacc()
    cvr = nc.declare_dram_parameter("cvr", [ntblk, 128, fwidth], f32, isOutput=False)
    ls = [nc.declare_dram_parameter(f"l{p}", [128, NP], f32, isOutput=False)
          for p in range(NPAIR)]
    wb = nc.declare_dram_parameter("wb", [NP, NP], f16, isOutput=False)
    bias = nc.declare_dram_parameter("bias", [NP, 1], f32, isOutput=False)
    hout = nc.declare_dram_parameter("hout", [NP, BL], f16, isOutput=True)

    with tile.TileContext(nc) as tc:
        with tc.tile_pool(name="const", bufs=1) as cpool, \
             tc.tile_pool(name="cv", bufs=min(CV_BUFS, ntblk)) as cvpool, \
             tc.tile_pool(name="state", bufs=t_steps + 8) as spool, \
             tc.tile_pool(name="scps", bufs=1, space="PSUM") as scps_pool, \
             tc.tile_pool(name="ps", bufs=7, space="PSUM") as ppool:
            l_t = []
            for p in range(NPAIR):
                lt = cpool.tile([128, NP], f32, tag=f"l{p}")
                nc.sync.dma_start(out=lt[:], in_=ls[p][:])
                l_t.append(lt)
            wb_t = cpool.tile([NP, NP], f16)
            nc.sync.dma_start(out=wb_t[:], in_=wb[:])
            bias_t = cpool.tile([NP, 1], f32)
            nc.sync.dma_start(out=bias_t[:], in_=bias[:])

            # Prologue: absorb each const-DMA semaphore with a dummy op so
            # later matmuls don't accumulate multiple sync waits.
            scratch_ps = scps_pool.tile([NP, NP], f32)
            for p in range(NPAIR):
                nc.tensor.matmul(scratch_ps[:], l_t[p][:NP, :], l_t[p][:NP, :],
                                 start=True, stop=True)
            nc.tensor.matmul(scratch_ps[:], wb_t[:], wb_t[:],
                             start=True, stop=True)
            scratch_sb = cpool.tile([NP, 1], f32)
            nc.scalar.activation(
                scratch_sb[:], bias_t[:], mybir.ActivationFunctionType.Tanh,
                bias=bias_t[:], scale=1.0,
            )

            cvmap = {}
            psq = {}
            state_prev = None
            for i in range(t_steps + AHEAD):
                if i < t_steps:
                    tblk, tq = divmod(i, TQ)
                    if tq == 0:
                        cv_tile = cvpool.tile([128, fwidth], f32)
                        nc.sync.dma_start(out=cv_tile[:], in_=cvr[tblk])
                        cvmap[tblk] = cv_tile
                    ps = ppool.tile([NP, BL], f32)
                    psq[i] = ps
                    base = tq * NPAIR * BL
                    for p in range(NPAIR):
                        nc.tensor.matmul(
                            ps[:], l_t[p][:],
                            cvmap[tblk][:, base + p * BL:base + (p + 1) * BL],
                            start=(p == 0), stop=(i == 0 and p == NPAIR - 1),
                        )
                s = i - AHEAD
                if s >= 0:
                    ps = psq.pop(s)
                    if s > 0:
                        nc.tensor.matmul(
                            ps[:], wb_t[:], state_prev[:],
                            start=False, stop=True,
                        )
                    st = spool.tile([NP, BL], f16)
                    nc.scalar.activation(
                        st[:], ps[:], mybir.ActivationFunctionType.Tanh,
                        bias=bias_t[:], scale=1.0,
                    )
                    state_prev = st
            nc.sync.dma_start(out=hout[:], in_=state_prev[:])
    nc.compile()
    return nc


def _pack_weights(W_ih, W_hh, b_ih, b_hh):
    Ls = []
    for p in range(NPAIR):
        L = np.zeros((128, NP), dtype=np.float32)
        for gl in range(2):
            g = 2 * p + gl
            for h in range(H):
                L[gl * 64:(gl + 1) * 64, g * 2 + h] = W_ih[h, :]
        Ls.append(L)
    WB = np.zeros((NP, NP), dtype=np.float16)
    w16 = W_hh.astype(np.float16)
    for g in range(NG):
        for h in range(H):
            for j in range(H):
                WB[g * 2 + h, g * 2 + j] = w16[j, h]
    biasv = np.tile((b_ih + b_hh).astype(np.float32), NG).reshape(NP, 1)
    return Ls, WB, np.ascontiguousarray(biasv)


def _pack_cv(cv, t_steps):
    # cv: [B, T, D] -> [core, tblk, (g_loc, d), (tq, pair, b_lo)]
    # b_local = pair*128 + g_loc*64 + b_lo
    ntblk = t_steps // TQ
    cv6 = cv.reshape(N_CORES, NPAIR, 2, BL, ntblk, TQ, D)  # core,p,gl,blo,tblk,tq,d
    cvR = cv6.transpose(0, 4, 2, 6, 5, 1, 3)               # core,tblk,gl,d,tq,p,blo
    return np.ascontiguousarray(
        cvR.reshape(N_CORES, ntblk, 128, TQ * NPAIR * BL))


def kernel(x=None, cv=None, W_ih=None, W_hh=None, b_ih=None, b_hh=None, **_):
    global LAST_EXEC_TIME_NS, LAST_RESULT
    from concourse.bass_utils import run_bass_kernel_spmd

    cv = np.ascontiguousarray(cv, dtype=np.float32)
    t_steps = cv.shape[1]
    if t_steps not in _PROGRAM_CACHE:
        _PROGRAM_CACHE[t_steps] = _build_program(t_steps)
    nc = _PROGRAM_CACHE[t_steps]

    Ls, WB, biasv = _pack_weights(
        np.asarray(W_ih, dtype=np.float32), np.asarray(W_hh, dtype=np.float32),
        np.asarray(b_ih, dtype=np.float32), np.asarray(b_hh, dtype=np.float32))
    cvR = _pack_cv(cv, t_steps)

    in_maps = [
        {"cvr": cvR[c], "wb": WB, "bias": biasv,
         **{f"l{p}": Ls[p] for p in range(NPAIR)}}
        for c in range(N_CORES)
    ]
    trace = bool(int(os.environ.get("KERNEL_TRACE", "0")))
    res = run_bass_kernel_spmd(nc, in_maps, list(range(N_CORES)), trace=trace)
    LAST_EXEC_TIME_NS = res.exec_time_ns
    LAST_RESULT = res

    out = np.empty((B, H), dtype=np.float32)
    for c in range(N_CORES):
        hc = res.results[c]["hout"].astype(np.float32)  # [(g,h)=16, b_lo=64]
        out[c * B_CORE:(c + 1) * B_CORE] = (
            hc.reshape(NG, H, BL).transpose(0, 2, 1).reshape(B_CORE, H)
        )
    return out
